# revision 5
# baseline (speedup 1.0000x reference)
"""Trainium2 Bass kernel for nn_ButterflyLayer.

Reference computation:
    h   = x @ w_in.T                       [B, 2048]
    h   = butterfly(h, a_pad, b_pad)       11 stages of paired rotations
    out = h @ w_out.T + b_out              [B, 2048]

Key algebraic facts used here:
  * The butterfly is a linear map B on the 2048-dim:  out = x @ (w_out @ B @ w_in).T + b.
  * B factors as (M (x) I_128) @ blockdiag(D_0..D_15) where
      - D_c (128x128) is the composition of stages 0..6 restricted to 128-chunk c
        (those stages never mix across 128-aligned chunks), and
      - stages 7..10 use one scalar coefficient per 128-chunk, so they act as a
        16x16 matrix M on chunk indices, identically for every position inside
        a chunk.
  * So W_eff = (w_out @ (M (x) I)) @ blockdiag(D) @ w_in, and the butterfly
    costs only a block-sparse (128-wide) contraction instead of a dense one.

Host prep is limited to O(dim^2) parameter/layout work: expanding the tiny
rotation params (a_pad/b_pad) into the D_c blocks, folding the 16x16 chunk mix
M into w_out, and permuting operands into PE-friendly tiled layouts so every
device load is one large fully-contiguous DMA. All O(batch*dim^2) compute runs
on the NeuronCores.

Device program (SPMD on 8 cores, 4 batch-groups x 2 out-column-groups):
  build:  g1[c]   = D_c^T-transform of the core's w_out' column slice   (32 mm)
          W_effT  = w_in-contraction of g1                              (512 mm)
  main:   outT    = W_effT^T @ xT (+ bias), streamed over batch         (1024 mm)
All matmuls use float32r (TF32-like fast fp32, 1 cycle/row at N=512).
"""

import sys

if "/opt/trn_rl_repo" not in sys.path:
    sys.path.insert(0, "/opt/trn_rl_repo")

import numpy as np

import concourse.bass as bass
import concourse.mybir as mybir
import concourse.tile as tile
from concourse import bacc
from concourse.bass import ts
from concourse.bass_utils import run_bass_kernel_spmd

DIM = 2048
LOG_DIM = 11
BATCH = 16384
N_CORES = 8
GB = 2                 # batch groups
GN = 4                 # output-column groups
MSH = BATCH // GB      # 4096 batch rows per core
NSL = DIM // GN        # 1024 output columns per core
P = 128                # partitions
NB = 512               # matmul moving free dim (one PSUM bank of fp32)
NCHUNK = DIM // P      # 16
N_NB = NSL // NB       # 2   (512-wide column blocks of the n-slice)
N_MB = MSH // NB       # 8   (512-wide batch blocks)
N_NT = NSL // P        # 8   (128-wide n tiles)
F32 = mybir.dt.float32
F32R = mybir.dt.float32r


# ---------------------------------------------------------------- host math

def _butterfly_dense(a_pad, b_pad, stages):
    """Dense matrix of the butterfly restricted to `stages` (float64).

    Returns Bm with butterfly(v) = Bm @ v for v in R^DIM.
    """
    x = np.eye(DIM, dtype=np.float64)  # rows: basis vectors
    for l in stages:
        bs = 1 << l
        nb = DIM // (2 * bs)
        a = a_pad[l, :nb].astype(np.float64)[None, :, None]
        b = b_pad[l, :nb].astype(np.float64)[None, :, None]
        xv = x.reshape(DIM, nb, 2, bs)
        x0 = xv[:, :, 0, :]
        x1 = xv[:, :, 1, :]
        top = a * x0 + b * x1
        bot = -b * x0 + a * x1
        x = np.stack([top, bot], axis=2).reshape(DIM, DIM)
    return x.T  # butterfly(I)[r] = Bm @ e_r, so butterfly(I) = Bm.T


def _host_prep(x, w_in, w_out, b_out, a_pad, b_pad):
    """Expand butterfly params; permute operands into tiled device layouts."""
    d_full = _butterfly_dense(a_pad, b_pad, range(7))           # blockdiag(D_c)
    m_full = _butterfly_dense(a_pad, b_pad, range(7, LOG_DIM))  # M (x) I_128
    m_small = np.ascontiguousarray(m_full[::P, ::P])            # [16, 16]

    # dstack[k, c*128+j] = D_c[k, j]  (one contiguous [128, 2048] tile row)
    d_arr = np.stack(
        [d_full[c * P:(c + 1) * P, c * P:(c + 1) * P] for c in range(NCHUNK)]
    )                                                           # [c, k, j]
    dstack = np.ascontiguousarray(
        d_arr.transpose(1, 0, 2).reshape(P, DIM)).astype(np.float32)

    # w_out' = w_out @ (M (x) I)
    w_out64 = w_out.astype(np.float64).reshape(DIM, NCHUNK, P)
    w_out_p = np.einsum("icj,cd->idj", w_out64, m_small).reshape(DIM, DIM)
    wopT = np.ascontiguousarray(w_out_p.T).astype(np.float32)   # [k, n]

    # w2[dt*128+p, ft*128+d] = w_in[ft*128+p, dt*128+d]
    w2 = np.ascontiguousarray(
        w_in.reshape(NCHUNK, P, NCHUNK, P).transpose(2, 1, 0, 3)
        .reshape(DIM, DIM))
    return dstack, wopT, w2


def _per_core_arrays(x, b_out, wopT, core):
    b, g = divmod(core, GN)
    # x2[mb*128+p, dt*512+m] = x[b*MSH + mb*512+m, dt*128+p]
    xs = x[b * MSH:(b + 1) * MSH, :]
    x2 = np.ascontiguousarray(
        xs.reshape(N_MB, NB, NCHUNK, P).transpose(0, 3, 2, 1)
        .reshape(N_MB * P, NCHUNK * NB))
    # g0t[nb*128+p, c*512+m] = wopT[c*128+p, g*NSL + nb*512+m]
    ws = wopT[:, g * NSL:(g + 1) * NSL]
    g0t = np.ascontiguousarray(
        ws.reshape(NCHUNK, P, N_NB, NB).transpose(2, 1, 0, 3)
        .reshape(N_NB * P, NCHUNK * NB))
    # bias2[p, nt] = b_out[g*NSL + nt*128 + p]
    bias2 = np.ascontiguousarray(
        b_out[g * NSL:(g + 1) * NSL].reshape(N_NT, P).T).astype(np.float32)
    return x2, g0t, bias2


# ------------------------------------------------------------- device build

def _build_nc():
    nc = bacc.Bacc("TRN2", target_bir_lowering=False, debug=False,
                   num_devices=N_CORES)

    xt = nc.dram_tensor("xt", [N_MB * P, NCHUNK * NB], F32R,
                        kind="ExternalInput")
    w2 = nc.dram_tensor("w2", [DIM, DIM], F32R, kind="ExternalInput")
    g0t = nc.dram_tensor("g0t", [N_NB * P, NCHUNK * NB], F32R,
                         kind="ExternalInput")
    dstk = nc.dram_tensor("dstk", [P, DIM], F32R, kind="ExternalInput")
    bias = nc.dram_tensor("bias", [P, N_NT], F32, kind="ExternalInput")
    # outt[(mb*N_NT+nt)*128+p, m] = out[b*MSH+mb*512+m, g*NSL+nt*128+p]
    outt = nc.dram_tensor("outt", [N_MB * N_NT * P, NB], F32,
                          kind="ExternalOutput")

    with tile.TileContext(nc) as tc:
        with (
            tc.tile_pool(name="geom", bufs=1) as geom,          # persistent
            tc.tile_pool(name="psum", bufs=8, space="PSUM") as psum,
        ):
            # --- persistent tiles
            dblk_sb = geom.tile([P, DIM], F32R)
            nc.sync.dma_start(out=dblk_sb, in_=dstk[:, :])
            bias_sb = geom.tile([P, N_NT], F32)
            nc.sync.dma_start(out=bias_sb, in_=bias[:, :])
            weff_sb = [geom.tile([P, NSL], F32R, name=f"weff{dt}")
                       for dt in range(NCHUNK)]
            # PE warmup while the first DMAs land (HAM ramp)
            wup = geom.tile([P, NB], mybir.dt.bfloat16, name="wup")
            nc.vector.memset(wup, 0.0)
            for _ in range(8):
                ptw = psum.tile([P, NB], F32, tag="ps")
                nc.tensor.matmul(ptw[:, :], wup[:, :P], wup, start=True,
                                 stop=True)

            # --- build W_effT = (w_in.T @ blockdiag(D).T @ w_out'.T)[:, n-slice]
            with tc.tile_pool(name="bld", bufs=2) as bld, \
                 tc.tile_pool(name="g1p", bufs=1) as g1p:
                g1_sb = [g1p.tile([P, NSL], F32R, name=f"g1_{c}")
                         for c in range(NCHUNK)]
                # g1[c] = D_c^T-transform of w_out'^T chunk c
                for nb in range(N_NB):
                    for c in range(NCHUNK):
                        g0c = bld.tile([P, NB], F32R, tag="g0", bufs=4)
                        nc.sync.dma_start(
                            out=g0c, in_=g0t[nb * P:(nb + 1) * P, ts(c, NB)])
                        pt = psum.tile([P, NB], F32, tag="ps")
                        nc.tensor.matmul(pt[:, :], dblk_sb[:, ts(c, P)], g0c,
                                         start=True, stop=True)
                        nc.any.tensor_copy(g1_sb[c][:, ts(nb, NB)], pt[:, :])
                # W_effT[dt] = sum_ft w_in[ft, dt].T @ g1[ft]
                for dt in range(NCHUNK):
                    wslab = bld.tile([P, DIM], F32R, tag="wslab", bufs=3)
                    nc.sync.dma_start(out=wslab,
                                      in_=w2[dt * P:(dt + 1) * P, :])
                    for nb in range(N_NB):
                        pt = psum.tile([P, NB], F32, tag="ps")
                        for ft in range(NCHUNK):
                            nc.tensor.matmul(pt[:, :], wslab[:, ts(ft, P)],
                                             g1_sb[ft][:, ts(nb, NB)],
                                             start=(ft == 0),
                                             stop=(ft == NCHUNK - 1))
                        nc.any.tensor_copy(weff_sb[dt][:, ts(nb, NB)], pt[:, :])

            # --- main: outT[nt, mb] = sum_dt W_effT[dt, nt].T @ xT[dt, mb] + bias
            with tc.tile_pool(name="mn", bufs=3) as mn, \
                 tc.tile_pool(name="ob", bufs=4) as ob:
                for mb in range(N_MB):
                    xs = mn.tile([P, NCHUNK * NB], F32R, tag="xs")
                    nc.sync.dma_start(out=xs, in_=xt[mb * P:(mb + 1) * P, :])
                    for nt in range(N_NT):
                        pt = psum.tile([P, NB], F32, tag="ps")
                        for dt in range(NCHUNK):
                            nc.tensor.matmul(pt[:, :],
                                             weff_sb[dt][:, ts(nt, P)],
                                             xs[:, ts(dt, NB)],
                                             start=(dt == 0),
                                             stop=(dt == NCHUNK - 1))
                        osb = ob.tile([P, NB], F32, tag="osb")
                        nc.scalar.activation(
                            osb, pt[:, :],
                            mybir.ActivationFunctionType.Identity,
                            bias=bias_sb[:, nt:nt + 1])
                        nc.sync.dma_start(
                            out=outt[(mb * N_NT + nt) * P:
                                     (mb * N_NT + nt + 1) * P, :],
                            in_=osb)

    nc.compile()
    return nc


_NC_CACHE = None


def _get_nc():
    global _NC_CACHE
    if _NC_CACHE is None:
        _NC_CACHE = _build_nc()
    return _NC_CACHE


# ------------------------------------------------------------------ driver

def _make_in_maps(x, w_in, w_out, b_out, a_pad, b_pad):
    dstack, wopT, w2 = _host_prep(x, w_in, w_out, b_out, a_pad, b_pad)
    in_maps = []
    for core in range(N_CORES):
        x2, g0t, bias2 = _per_core_arrays(x, b_out, wopT, core)
        in_maps.append({
            "xt": x2,
            "w2": w2,
            "g0t": g0t,
            "dstk": dstack,
            "bias": bias2,
        })
    return in_maps


def _assemble(results):
    out = np.empty((BATCH, DIM), dtype=np.float32)
    for core in range(N_CORES):
        b, g = divmod(core, GN)
        # outt rows [(mb*N_NT+nt)*128+p], cols m
        arr = results[core]["outt"].reshape(N_MB, N_NT, P, NB)
        # -> [mb, m, nt, p]
        out[b * MSH:(b + 1) * MSH, g * NSL:(g + 1) * NSL] = \
            arr.transpose(0, 3, 1, 2).reshape(MSH, NSL)
    return out


def kernel(x, w_in, w_out, b_out, a_pad, b_pad, _trace=False):
    nc = _get_nc()
    in_maps = _make_in_maps(x, w_in, w_out, b_out, a_pad, b_pad)
    res = run_bass_kernel_spmd(nc, in_maps, core_ids=list(range(N_CORES)),
                               trace=_trace)
    out = _assemble(res.results)
    if _trace:
        kernel.last_result = res
    return out


# revision 6
# speedup vs baseline: 86439.7206x; 86439.7206x over previous
"""Trainium2 Bass kernel for nn_ButterflyLayer.

Reference computation:
    h   = x @ w_in.T                       [B, 2048]
    h   = butterfly(h, a_pad, b_pad)       11 stages of paired rotations
    out = h @ w_out.T + b_out              [B, 2048]

Key algebraic facts used here:
  * The butterfly is a linear map B on the 2048-dim:  out = x @ (w_out @ B @ w_in).T + b.
  * B factors as (M (x) I_128) @ blockdiag(D_0..D_15) where
      - D_c (128x128) is the composition of stages 0..6 restricted to 128-chunk c
        (those stages never mix across 128-aligned chunks), and
      - stages 7..10 use one scalar coefficient per 128-chunk, so they act as a
        16x16 matrix M on chunk indices, identically for every position inside
        a chunk.
  * So W_eff = (w_out @ (M (x) I)) @ blockdiag(D) @ w_in, and the butterfly
    costs only a block-sparse (128-wide) contraction instead of a dense one.

Host prep is limited to O(dim^2) parameter/layout work: expanding the tiny
rotation params (a_pad/b_pad) into the D_c blocks, folding the 16x16 chunk mix
M into w_out, and permuting operands into PE-friendly tiled layouts so every
device load is one large fully-contiguous DMA. All O(batch*dim^2) compute runs
on the NeuronCores.

Device program (SPMD on 8 cores, 2 batch-groups x 4 out-column-groups):
  build:  g1[c]   = D_c^T-transform of the core's w_out' column slice   (32 mm)
          W_effT  = w_in-contraction of g1                              (512 mm)
  main:   outT    = W_effT^T @ xT (+ bias), streamed over batch         (1024 mm)
All matmuls use float32r (TF32-like fast fp32, 1 cycle/row at N=512).
"""

import sys

if "/opt/trn_rl_repo" not in sys.path:
    sys.path.insert(0, "/opt/trn_rl_repo")

import numpy as np

import concourse.bass as bass
import concourse.mybir as mybir
import concourse.tile as tile
from concourse import bacc
from concourse.bass import ts
from concourse.bass_utils import run_bass_kernel_spmd

DIM = 2048
LOG_DIM = 11
BATCH = 16384
N_CORES = 8
GB = 2                 # batch groups
GN = 4                 # output-column groups
MSH = BATCH // GB      # 4096 batch rows per core
NSL = DIM // GN        # 1024 output columns per core
P = 128                # partitions
NB = 512               # matmul moving free dim (one PSUM bank of fp32)
NCHUNK = DIM // P      # 16
N_NB = NSL // NB       # 2   (512-wide column blocks of the n-slice)
N_MB = MSH // NB       # 8   (512-wide batch blocks)
N_NT = NSL // P        # 8   (128-wide n tiles)
F32 = mybir.dt.float32
F32R = mybir.dt.float32r


# ---------------------------------------------------------------- host math

def _butterfly_dense(a_pad, b_pad, stages):
    """Dense matrix of the butterfly restricted to `stages` (float64).

    Returns Bm with butterfly(v) = Bm @ v for v in R^DIM.
    """
    x = np.eye(DIM, dtype=np.float64)  # rows: basis vectors
    for l in stages:
        bs = 1 << l
        nb = DIM // (2 * bs)
        a = a_pad[l, :nb].astype(np.float64)[None, :, None]
        b = b_pad[l, :nb].astype(np.float64)[None, :, None]
        xv = x.reshape(DIM, nb, 2, bs)
        x0 = xv[:, :, 0, :]
        x1 = xv[:, :, 1, :]
        top = a * x0 + b * x1
        bot = -b * x0 + a * x1
        x = np.stack([top, bot], axis=2).reshape(DIM, DIM)
    return x.T  # butterfly(I)[r] = Bm @ e_r, so butterfly(I) = Bm.T


def _host_prep(x, w_in, w_out, b_out, a_pad, b_pad):
    """Expand butterfly params; permute operands into tiled device layouts."""
    d_full = _butterfly_dense(a_pad, b_pad, range(7))           # blockdiag(D_c)
    m_full = _butterfly_dense(a_pad, b_pad, range(7, LOG_DIM))  # M (x) I_128
    m_small = np.ascontiguousarray(m_full[::P, ::P])            # [16, 16]

    # dstack[k, c*128+j] = D_c[k, j]  (one contiguous [128, 2048] tile row)
    d_arr = np.stack(
        [d_full[c * P:(c + 1) * P, c * P:(c + 1) * P] for c in range(NCHUNK)]
    )                                                           # [c, k, j]
    dstack = np.ascontiguousarray(
        d_arr.transpose(1, 0, 2).reshape(P, DIM)).astype(np.float32)

    # w_out' = w_out @ (M (x) I)
    w_out64 = w_out.astype(np.float64).reshape(DIM, NCHUNK, P)
    w_out_p = np.einsum("icj,cd->idj", w_out64, m_small).reshape(DIM, DIM)
    wopT = np.ascontiguousarray(w_out_p.T).astype(np.float32)   # [k, n]

    # w2[dt*128+p, ft*128+d] = w_in[ft*128+p, dt*128+d]
    w2 = np.ascontiguousarray(
        w_in.reshape(NCHUNK, P, NCHUNK, P).transpose(2, 1, 0, 3)
        .reshape(DIM, DIM))
    return dstack, wopT, w2


def _per_core_arrays(x, b_out, wopT, core):
    b, g = divmod(core, GN)
    # x2[mb*128+p, dt*512+m] = x[b*MSH + mb*512+m, dt*128+p]
    xs = x[b * MSH:(b + 1) * MSH, :]
    x2 = np.ascontiguousarray(
        xs.reshape(N_MB, NB, NCHUNK, P).transpose(0, 3, 2, 1)
        .reshape(N_MB * P, NCHUNK * NB))
    # g0t[nb*128+p, c*512+m] = wopT[c*128+p, g*NSL + nb*512+m]
    ws = wopT[:, g * NSL:(g + 1) * NSL]
    g0t = np.ascontiguousarray(
        ws.reshape(NCHUNK, P, N_NB, NB).transpose(2, 1, 0, 3)
        .reshape(N_NB * P, NCHUNK * NB))
    # bias2[p, nt] = b_out[g*NSL + nt*128 + p]
    bias2 = np.ascontiguousarray(
        b_out[g * NSL:(g + 1) * NSL].reshape(N_NT, P).T).astype(np.float32)
    return x2, g0t, bias2


# ------------------------------------------------------------- device build

def _build_nc():
    nc = bacc.Bacc("TRN2", target_bir_lowering=False, debug=False,
                   num_devices=N_CORES)

    xt = nc.dram_tensor("xt", [N_MB * P, NCHUNK * NB], F32R,
                        kind="ExternalInput")
    w2 = nc.dram_tensor("w2", [DIM, DIM], F32R, kind="ExternalInput")
    g0t = nc.dram_tensor("g0t", [N_NB * P, NCHUNK * NB], F32R,
                         kind="ExternalInput")
    dstk = nc.dram_tensor("dstk", [P, DIM], F32R, kind="ExternalInput")
    bias = nc.dram_tensor("bias", [P, N_NT], F32, kind="ExternalInput")
    # outt[(mb*N_NT+nt)*128+p, m] = out[b*MSH+mb*512+m, g*NSL+nt*128+p]
    outt = nc.dram_tensor("outt", [N_MB * N_NT * P, NB], F32,
                          kind="ExternalOutput")

    with tile.TileContext(nc) as tc:
        with (
            tc.tile_pool(name="geom", bufs=1) as geom,          # persistent
            tc.tile_pool(name="psum", bufs=8, space="PSUM") as psum,
        ):
            # --- persistent tiles
            dblk_sb = geom.tile([P, DIM], F32R)
            nc.sync.dma_start(out=dblk_sb, in_=dstk[:, :])
            bias_sb = geom.tile([P, N_NT], F32)
            nc.sync.dma_start(out=bias_sb, in_=bias[:, :])
            weff_sb = [geom.tile([P, NSL], F32R, name=f"weff{dt}")
                       for dt in range(NCHUNK)]
            # PE warmup while the first DMAs land (HAM ramp)
            wup = geom.tile([P, NB], mybir.dt.bfloat16, name="wup")
            nc.vector.memset(wup, 0.0)
            for _ in range(8):
                ptw = psum.tile([P, NB], F32, tag="ps")
                nc.tensor.matmul(ptw[:, :], wup[:, :P], wup, start=True,
                                 stop=True)

            # --- build W_effT = (w_in.T @ blockdiag(D).T @ w_out'.T)[:, n-slice]
            with tc.tile_pool(name="bld", bufs=2) as bld, \
                 tc.tile_pool(name="g1p", bufs=1) as g1p:
                g1_sb = [g1p.tile([P, NSL], F32R, name=f"g1_{c}")
                         for c in range(NCHUNK)]
                # g1[c] = D_c^T-transform of w_out'^T chunk c
                for nb in range(N_NB):
                    for c in range(NCHUNK):
                        g0c = bld.tile([P, NB], F32R, tag="g0", bufs=4)
                        nc.sync.dma_start(
                            out=g0c, in_=g0t[nb * P:(nb + 1) * P, ts(c, NB)])
                        pt = psum.tile([P, NB], F32, tag="ps")
                        nc.tensor.matmul(pt[:, :], dblk_sb[:, ts(c, P)], g0c,
                                         start=True, stop=True)
                        nc.any.tensor_copy(g1_sb[c][:, ts(nb, NB)], pt[:, :])
                # W_effT[dt] = sum_ft w_in[ft, dt].T @ g1[ft]
                for dt in range(NCHUNK):
                    wslab = bld.tile([P, DIM], F32R, tag="wslab", bufs=3)
                    nc.sync.dma_start(out=wslab,
                                      in_=w2[dt * P:(dt + 1) * P, :])
                    for nb in range(N_NB):
                        pt = psum.tile([P, NB], F32, tag="ps")
                        for ft in range(NCHUNK):
                            nc.tensor.matmul(pt[:, :], wslab[:, ts(ft, P)],
                                             g1_sb[ft][:, ts(nb, NB)],
                                             start=(ft == 0),
                                             stop=(ft == NCHUNK - 1))
                        nc.any.tensor_copy(weff_sb[dt][:, ts(nb, NB)], pt[:, :])

            # --- main: outT[nt, mb] = sum_dt W_effT[dt, nt].T @ xT[dt, mb] + bias
            with tc.tile_pool(name="mn", bufs=3) as mn, \
                 tc.tile_pool(name="ob", bufs=4) as ob:
                for mb in range(N_MB):
                    xs = mn.tile([P, NCHUNK * NB], F32R, tag="xs")
                    nc.sync.dma_start(out=xs, in_=xt[mb * P:(mb + 1) * P, :])
                    for nt in range(N_NT):
                        pt = psum.tile([P, NB], F32, tag="ps")
                        for dt in range(NCHUNK):
                            nc.tensor.matmul(pt[:, :],
                                             weff_sb[dt][:, ts(nt, P)],
                                             xs[:, ts(dt, NB)],
                                             start=(dt == 0),
                                             stop=(dt == NCHUNK - 1))
                        osb = ob.tile([P, NB], F32, tag="osb")
                        nc.scalar.activation(
                            osb, pt[:, :],
                            mybir.ActivationFunctionType.Identity,
                            bias=bias_sb[:, nt:nt + 1])
                        nc.sync.dma_start(
                            out=outt[(mb * N_NT + nt) * P:
                                     (mb * N_NT + nt + 1) * P, :],
                            in_=osb)

    nc.compile()
    return nc


_NC_CACHE = None


def _get_nc():
    global _NC_CACHE
    if _NC_CACHE is None:
        _NC_CACHE = _build_nc()
    return _NC_CACHE


# ------------------------------------------------------------------ driver

def _make_in_maps(x, w_in, w_out, b_out, a_pad, b_pad):
    dstack, wopT, w2 = _host_prep(x, w_in, w_out, b_out, a_pad, b_pad)
    in_maps = []
    for core in range(N_CORES):
        x2, g0t, bias2 = _per_core_arrays(x, b_out, wopT, core)
        in_maps.append({
            "xt": x2,
            "w2": w2,
            "g0t": g0t,
            "dstk": dstack,
            "bias": bias2,
        })
    return in_maps


def _assemble(results):
    out = np.empty((BATCH, DIM), dtype=np.float32)
    for core in range(N_CORES):
        b, g = divmod(core, GN)
        # outt rows [(mb*N_NT+nt)*128+p], cols m
        arr = results[core]["outt"].reshape(N_MB, N_NT, P, NB)
        # -> [mb, m, nt, p]
        out[b * MSH:(b + 1) * MSH, g * NSL:(g + 1) * NSL] = \
            arr.transpose(0, 3, 1, 2).reshape(MSH, NSL)
    return out


def kernel(x, w_in, w_out, b_out, a_pad, b_pad, _trace=False):
    x = np.asarray(x, dtype=np.float32)
    w_in = np.asarray(w_in, dtype=np.float32)
    w_out = np.asarray(w_out, dtype=np.float32)
    b_out = np.asarray(b_out, dtype=np.float32)
    a_pad = np.asarray(a_pad, dtype=np.float32)
    b_pad = np.asarray(b_pad, dtype=np.float32)
    nc = _get_nc()
    in_maps = _make_in_maps(x, w_in, w_out, b_out, a_pad, b_pad)
    res = run_bass_kernel_spmd(nc, in_maps, core_ids=list(range(N_CORES)),
                               trace=_trace)
    out = _assemble(res.results)
    if _trace:
        kernel.last_result = res
    return out
